# revision 19
# baseline (speedup 1.0000x reference)
"""CVRP decoder (3-layer transformer + scatter) on 8 trn2 NeuronCores.

Self-contained: hardcodes shapes/sharding for
  nn_CVRP_Decoder (B=512, SEQ=102, EMBED=256, HEADS=16, DK=16, FF=1024, L=3).

Strategy: pure data parallel over batch (64 rows/core). Per core the
whole network runs feature-major ([256->2x128 partitions, tokens]) in
groups of 8 batch rows (816 tokens); attention is row-local. All
matmuls fp8 DoubleRow (fp32 PSUM accumulate), residual stream fp32.

Attention (all fp8 DR):
 - scores: S^T for all 16 heads of a row in 2 matmuls via a
   block-diagonal q layout qd16 [128, 2, row, 16 slots, SEQ] against
   lhsT = k8 (feature-major fp8).
 - exp: ACT psum -> fp8 exps [SEQ, 16, SEQ] (2 ops/row); scores are
   tiny (|s/4| < 1) so exps are in [0.4, 2.4], perfect fp8 range.
 - attnV: per head-pair p one DR matmul, lhsT = v8d [SEQ, pair, 2, 32]
   ([V_2p | 0]j0, [0 | V_2p+1]j1), out = o block [32, SEQ] in
   apscs[32*(p%4):+32, 0, p//4]; matching ones-matmuls (shared const
   lhsT, value 64) produce the softmax denominators in
   apscs[..., 1, ...] at the SAME partitions (DVE ops require equal
   start partitions, so denominators must be born replicated).
 - normalize: rcp(denoms) + mul -> onrm fp8 [128, 2, GR, SEQ]
   (= o/8, consumed by Wc packed [128, 3, 2, 256] at x8).
Final scatter into [64, 2002] via GPSIMD local_scatter of hi/lo bf16
halves, summed in fp32.
"""

import sys

if "/opt/trn_rl_repo" not in sys.path:
    sys.path.insert(0, "/opt/trn_rl_repo")

import numpy as np
import ml_dtypes

B = 512
SEQ = 102
EMBED = 256
HEADS = 16
DK = 16
FF = 1024
LAYERS = 3
N_CORES = 8
RPC = B // N_CORES        # rows per core = 64
GR = 8                    # rows per group
GROUPS = RPC // GR        # 8
GT = GR * SEQ             # tokens per group = 816
NH = 2                    # N-halves per group (408 each)
NSZ = GT // NH            # 408
P1 = 1001
OUT_W = 2 * P1            # 2002

_prog_cache = {}


def _pack_k_major(w, ko):
    # [K, M] -> [128, ko, M] with K = ko*128 split as (ko ki)
    K, M = w.shape
    assert K == ko * 128
    return np.ascontiguousarray(w.reshape(ko, 128, M).transpose(1, 0, 2))


def _host_weights(inp):
    f32 = np.float32
    f8 = ml_dtypes.float8_e4m3

    def g(name):
        return np.asarray(inp[name], dtype=f32)

    wq, wk, wv = g("Wq"), g("Wk"), g("Wv")      # [3, 256, 256]
    wc, bc = g("Wc"), g("bc")                   # [3, 256, 256], [3, 256]
    w1, b1 = g("W1"), g("b1")                   # [3, 256, 1024], [3, 1024]
    w2, b2 = g("W2"), g("b2")                   # [3, 1024, 256], [3, 256]

    def stack_layers(ws, ko, dt):
        return np.stack([_pack_k_major(w, ko) for w in ws], axis=1).astype(dt)
        # -> [128, 3, ko, M]

    # fp8 scale folds (all exact, host-side):
    #   weights x8 into fp8; q,k carry x8 each -> exp scale /64
    #   v8d = 8v; ones value 64 -> onrm = attn/8; Wc psum = attn*wc
    #   residual bias = bc + b2 (b2 folded out of FF2)
    #   FF1 psum = 8*o1'*w1, b1' = 8*(b1 - b2@W1) corrects the folded b2
    #   FF2 psum = (8h)(8w2) -> epilogue multiplies 1/64
    out = {}
    out["wq"] = stack_layers(wq * 8, 2, f8)     # [128, 3, 2, 256]
    out["wk"] = stack_layers(wk * 8, 2, f8)
    out["wv"] = stack_layers(wv * 8, 2, f8)
    # Wc packed for the attention output layout: o lives on 64 partitions,
    # 4 couple-slots; at (partition P, slot S): hc = P//16, head
    # h = 4S + 2*(hc%2) + hc//2, dim P%16
    wc2 = np.zeros((64, 3, 4, 256), f32)
    for P in range(64):
        for S in range(4):
            hc = P // 16          # = 2*eo + j
            h = 4 * S + 2 * (hc // 2) + hc % 2
            fo = 16 * h + (P % 16)
            wc2[P, :, S, :] = wc[:, fo, :] * 8
    out["wc"] = wc2.astype(f8)                  # [64, 3, 4, 256]
    out["w1"] = stack_layers(w1 * 8, 2, f8)     # [128, 3, 2, 1024]
    out["w2"] = stack_layers(w2 * 8, 8, f8)     # [128, 3, 8, 256]
    bcb2 = bc + b2
    b1f = 8.0 * (b1 - np.einsum("ld,ldf->lf", b2, w1))
    out["bc"] = np.ascontiguousarray(
        bcb2.reshape(3, 2, 128).transpose(2, 0, 1)).astype(f32)  # [128, 3, 2]
    out["b1"] = np.ascontiguousarray(
        b1f.reshape(3, 8, 128).transpose(2, 0, 1)).astype(f32)   # [128, 3, 8]
    out["wnv"] = _pack_k_major(g("W_nv") * 8, 2).astype(f8)     # [128, 2, 256]
    out["wv2"] = _pack_k_major(g("W_v") * 8, 2).astype(f8)
    out["bnv"] = np.ascontiguousarray(
        g("b_nv").reshape(2, 128).T).astype(f32)                # [128, 2]
    out["bv2"] = np.ascontiguousarray(g("b_v").reshape(2, 128).T).astype(f32)
    out["wf"] = np.ascontiguousarray(
        g("Wf").reshape(2, 128).T).astype(f32)                  # [128, 2]
    return out


def _build_program():
    import concourse.bass as bass
    import concourse.tile as tile
    from concourse import bacc, mybir

    f32 = mybir.dt.float32
    bf16 = mybir.dt.bfloat16

    nc = bacc.Bacc("TRN2", target_bir_lowering=False, debug=False,
                   num_devices=N_CORES)

    def din(name, shape, dt=f32):
        return nc.declare_dram_parameter(name, list(shape), dt, isOutput=False)

    f8 = mybir.dt.float8e4

    x_d = din("x", [128, 2, RPC, SEQ])
    mask_d = din("mask", [RPC, SEQ])
    idx_d = din("idx", [RPC, 100], mybir.dt.int16)
    wq_d = din("wq", [128, 3, 2, 256], f8)
    wk_d = din("wk", [128, 3, 2, 256], f8)
    wv_d = din("wv", [128, 3, 2, 256], f8)
    wc_d = din("wc", [64, 3, 4, 256], f8)
    w1_d = din("w1", [128, 3, 2, 1024], f8)
    w2_d = din("w2", [128, 3, 8, 256], f8)
    bc_d = din("bc", [128, 3, 2])
    b1_d = din("b1", [128, 3, 8])
    wnv_d = din("wnv", [128, 2, 256], f8)
    wv2_d = din("wv2", [128, 2, 256], f8)
    bnv_d = din("bnv", [128, 2])
    bv2_d = din("bv2", [128, 2])
    wf_d = din("wf", [128, 2])
    out_d = nc.declare_dram_parameter("out", [RPC, OUT_W], f32, isOutput=True)

    with tile.TileContext(nc) as tc:
        wpool = tc.alloc_tile_pool(name="w", bufs=1)
        xpool = tc.alloc_tile_pool(name="x", bufs=3)
        apool = tc.alloc_tile_pool(name="a", bufs=2)
        vpool = tc.alloc_tile_pool(name="v", bufs=1)
        hdpool = tc.alloc_tile_pool(name="hd", bufs=1)
        espool = tc.alloc_tile_pool(name="es", bufs=3)
        lin_ps = tc.alloc_tile_pool(name="lps", bufs=2, space="PSUM")
        s_ps = tc.alloc_tile_pool(name="sps", bufs=1, space="PSUM")
        a_ps = tc.alloc_tile_pool(name="aps", bufs=2, space="PSUM")

        # ---- persistent weights ----
        # small first-needed tensors on sync; bulk weights on gpsimd so the
        # first group's x load isn't queued behind ~2.6MB of weight DMA
        def wtile(dram, shape, dt, tag, eng=None):
            t = wpool.tile(list(shape), dt, tag=tag)
            (eng or nc.sync).dma_start(out=t[:], in_=dram[:])
            return t

        wnv = wtile(wnv_d, [128, 2, 256], f8, "wnv")
        wv2 = wtile(wv2_d, [128, 2, 256], f8, "wv2")
        bnv = wtile(bnv_d, [128, 2], f32, "bnv")
        bv2 = wtile(bv2_d, [128, 2], f32, "bv2")
        wq = wtile(wq_d, [128, 3, 2, 256], f8, "wq", nc.gpsimd)
        wk = wtile(wk_d, [128, 3, 2, 256], f8, "wk", nc.gpsimd)
        wv = wtile(wv_d, [128, 3, 2, 256], f8, "wv", nc.gpsimd)
        wc = wtile(wc_d, [64, 3, 4, 256], f8, "wc", nc.gpsimd)
        w1 = wtile(w1_d, [128, 3, 2, 1024], f8, "w1", nc.gpsimd)
        w2 = wtile(w2_d, [128, 3, 8, 256], f8, "w2", nc.gpsimd)
        bc = wtile(bc_d, [128, 3, 2], f32, "bc", nc.gpsimd)
        b1 = wtile(b1_d, [128, 3, 8], f32, "b1", nc.gpsimd)
        wf = wtile(wf_d, [128, 2], f32, "wf", nc.gpsimd)
        mask_sb = wtile(mask_d, [RPC, SEQ], f32, "mask", nc.gpsimd)
        idx_sb = wtile(idx_d, [RPC, 100], mybir.dt.int16, "idx", nc.gpsimd)

        lg = wpool.tile([RPC, SEQ], f32)

        # constant ones lhsTs for the denominator matmuls (couple layout:
        # head (eo, j) of a couple sits at out partitions 16*(2j+eo)); value
        # 64 so denominators land at the same partitions as the o blocks
        ones_eo = []
        for eo in range(2):
            t = wpool.tile([SEQ, 2, 64], f8, tag=f"ones{eo}")
            nc.vector.memset(t[:], 0.0)
            for j in range(2):
                m = 16 * (2 * eo + j)
                nc.vector.memset(t[:, j, m : m + 16], 64.0)
            ones_eo.append(t)

        # block-diagonal q layout [128(h%8 blocks), 2(h//8 plane), row, slot, n];
        # only the diagonal blocks are ever DMA-written, zeros persist
        qds = [hdpool.tile([128, 2, 4, HEADS, SEQ], f8, name=f"qd{i}",
                           tag=f"qd{i}") for i in range(2)]
        nc.vector.memset(qds[0][:], 0.0)
        nc.gpsimd.memset(qds[1][:], 0.0)

        # v8d ring: lhsT for attnV couples, [SEQ, couple, eo, j, m(64)];
        # V blocks rewritten per use, zero padding persists. Head
        # h = 4c + 2eo + j sits at (c, eo, j, m=16*(2eo+j)..+16).
        NV = 10
        v8ds = [vpool.tile([SEQ, 4, 2, 2, 64], f8, name=f"v8d{i}", tag=f"v8d{i}")
                for i in range(NV)]
        for i, t in enumerate(v8ds):
            (nc.vector if i % 2 else nc.gpsimd).memset(t[:], 0.0)

        for g in range(GROUPS):
            b0 = g * GR
            # ---- load x group into feature-major reordered layout ----
            xt = xpool.tile([128, 2, GR, SEQ], f32, tag="xt")
            for ko in range(2):
                nc.sync.dma_start(out=xt[:, ko], in_=x_d[:, ko, b0 : b0 + GR, :])
            # cast x8 from the raw load first (overlaps the pos projections),
            # then patch the two projected columns below
            x8 = apool.tile([128, 2, GR, SEQ], f8, tag="x8")
            for nh in range(NH):
                rr = slice(nh * 4, nh * 4 + 4)
                nc.scalar.copy(out=x8[:, :, rr], in_=xt[:, :, rr])
            # positions 0 / 51 hold raw tokens 50 / 101; project in place
            # (fp8 DoubleRow; psum = 8*(x@W) -> scale 1/8 in the bias stage)
            for w_t, b_t, pos in ((wnv, bnv, 0), (wv2, bv2, 51)):
                ps = lin_ps.tile([128, 2, GR], f32, tag="lin")
                for mo in range(2):
                    nc.tensor.matmul(
                        out=ps[:, mo, :],
                        lhsT=w_t[:, :, mo * 128 : (mo + 1) * 128],
                        rhs=x8[:, :, :, pos],
                        start=True, stop=True,
                        perf_mode=mybir.MatmulPerfMode.DoubleRow)
                for mo in range(2):
                    nc.scalar.activation(
                        out=xt[:, mo, :, pos],
                        in_=ps[:, mo, :],
                        func=mybir.ActivationFunctionType.Identity,
                        bias=b_t[:, mo : mo + 1], scale=0.125)
            for pos in (0, 51):
                nc.scalar.copy(out=x8[:, :, :, pos], in_=xt[:, :, :, pos])

            # ---- layers ----
            # later layers' x8 are cast chunk-by-chunk inside the previous
            # layer's FF2 loop so the PE never waits at the layer boundary.
            for l in range(LAYERS):
                # Q, K projections (feature-major, fp8 DoubleRow over both
                # K-halves; psum carries x8 from the weight scale)
                q8 = apool.tile([128, 2, GR, SEQ], f8, tag="q8")
                k8 = apool.tile([128, 2, GR, SEQ], f8, tag="k8")
                for nh in range(NH):
                    rr = slice(nh * 4, nh * 4 + 4)
                    for w_t, o_t in ((wq, q8), (wk, k8)):
                        for mo in range(2):
                            ps = lin_ps.tile([128, NSZ], f32, tag="lin")
                            nc.tensor.matmul(
                                out=ps[:],
                                lhsT=w_t[:, l, :, mo * 128 : (mo + 1) * 128],
                                rhs=x8[:, :, rr],
                                start=True, stop=True,
                                perf_mode=mybir.MatmulPerfMode.DoubleRow)
                            nc.vector.tensor_copy(out=o_t[:, mo, rr], in_=ps[:])

                # V token-major per row (fp8 DoubleRow; psum carries x8);
                # written straight into the v8d couple layout: head
                # (c, eo, j) -> vt[:, c, eo, j, 16*(2j+eo)]; per-eo slices
                # [:, :, eo, :, 16*eo:+16] hit exactly those positions
                vbfs = []
                for b in range(GR):
                    ps = lin_ps.tile([SEQ, 4, 2, 2, 16], f32, tag="lin")
                    nc.tensor.matmul(
                        out=ps[:],
                        lhsT=x8[:, :, b, :],
                        rhs=wv[:, l],
                        start=True, stop=True,
                        perf_mode=mybir.MatmulPerfMode.DoubleRow)
                    vt = v8ds[(g * LAYERS * GR + l * GR + b) % NV]
                    for eo in range(2):
                        for j in range(2):
                            m = 16 * (2 * eo + j)
                            nc.vector.tensor_copy(
                                out=vt[:, :, eo, j, m : m + 16],
                                in_=ps[:, :, eo, j, :])
                    vbfs.append(vt)

                onrm = apool.tile([64, 4, GR, SEQ], f8, tag="onrm")
                out1 = apool.tile([128, 2, GR, SEQ], f32, tag="out1")
                o18 = apool.tile([128, 2, GR, SEQ], f8, tag="o18")
                h8 = apool.tile([128, 8, GR, SEQ], f8, tag="h8", bufs=1)
                x8n = apool.tile([128, 2, GR, SEQ], f8, tag="x8")

                # linear-stack emitters (per nh column chunk); for nh=0 these
                # are interleaved into half 1's attention loop so the PE and
                # DVE/ACT stay busy through both phases
                def emit_wc(nh):
                    rr = slice(nh * 4, nh * 4 + 4)
                    for mo in range(2):
                        ps = lin_ps.tile([128, NSZ], f32, tag="lin")
                        for u in range(2):
                            nc.tensor.matmul(
                                out=ps[:],
                                lhsT=wc[:, l, 2 * u : 2 * u + 2,
                                        mo * 128 : (mo + 1) * 128],
                                rhs=onrm[:, 2 * u : 2 * u + 2, rr],
                                start=(u == 0), stop=(u == 1),
                                perf_mode=mybir.MatmulPerfMode.DoubleRow)
                        nc.vector.scalar_tensor_tensor(
                            out=out1[:, mo, rr], in0=ps[:],
                            scalar=bc[:, l, mo : mo + 1],
                            in1=xt[:, mo, rr],
                            op0=mybir.AluOpType.add, op1=mybir.AluOpType.add)
                        nc.scalar.copy(out=o18[:, mo, rr],
                                       in_=out1[:, mo, rr])

                def emit_ff1(nh, mos):
                    rr = slice(nh * 4, nh * 4 + 4)
                    for mo in mos:
                        ps = lin_ps.tile([128, NSZ], f32, tag="lin")
                        nc.tensor.matmul(
                            out=ps[:],
                            lhsT=w1[:, l, :, mo * 128 : (mo + 1) * 128],
                            rhs=o18[:, :, rr],
                            start=True, stop=True,
                            perf_mode=mybir.MatmulPerfMode.DoubleRow)
                        if mo % 2 == 0:
                            nc.scalar.activation(
                                out=h8[:, mo, rr], in_=ps[:],
                                func=mybir.ActivationFunctionType.Relu,
                                bias=b1[:, l, mo : mo + 1], scale=1.0)
                        else:
                            nc.vector.tensor_scalar(
                                out=h8[:, mo, rr], in0=ps[:],
                                scalar1=b1[:, l, mo : mo + 1], scalar2=0.0,
                                op0=mybir.AluOpType.add, op1=mybir.AluOpType.max)

                def emit_ff2(nh):
                    rr = slice(nh * 4, nh * 4 + 4)
                    for mo in range(2):
                        ps = lin_ps.tile([128, NSZ], f32, tag="lin")
                        for ko in range(4):
                            nc.tensor.matmul(
                                out=ps[:],
                                lhsT=w2[:, l, 2 * ko : 2 * ko + 2,
                                        mo * 128 : (mo + 1) * 128],
                                rhs=h8[:, 2 * ko : 2 * ko + 2, rr],
                                start=(ko == 0), stop=(ko == 3),
                                perf_mode=mybir.MatmulPerfMode.DoubleRow)
                        nc.vector.scalar_tensor_tensor(
                            out=xt[:, mo, rr], in0=ps[:],
                            scalar=1.0 / 64.0,
                            in1=out1[:, mo, rr],
                            op0=mybir.AluOpType.mult, op1=mybir.AluOpType.add)
                    if l < LAYERS - 1:
                        nc.scalar.copy(out=x8n[:, :, rr], in_=xt[:, :, rr])

                fill = {(1, 0): lambda: emit_wc(0),
                        (1, 1): lambda: emit_ff1(0, range(0, 4)),
                        (1, 2): lambda: emit_ff1(0, range(4, 8)),
                        (1, 3): lambda: emit_ff2(0)}
                dmae = [nc.sync, nc.gpsimd]
                for half in range(2):
                    qd = qds[(g * LAYERS * 2 + l * 2 + half) % 2]
                    hrr = slice(half * 4, half * 4 + 4)
                    for s in range(HEADS):
                        dmae[s % 2].dma_start(
                            out=qd[16 * (s % 8) : 16 * (s % 8) + 16, s // 8, :, s, :],
                            in_=q8[16 * (s % 8) : 16 * (s % 8) + 16, s // 8, hrr])
                    for bi in range(4):
                        b = half * 4 + bi
                        exps = espool.tile([SEQ, HEADS, SEQ], f8, tag="exps")
                        apscs = a_ps.tile([64, 2, 4, SEQ], f32, tag="apscs",
                                          bufs=1)
                        for hh in range(2):
                            # psum matmul regions must stay within 2KB banks:
                            # pad the inner dim to 128 so each j-chunk is
                            # exactly one bank
                            sc = s_ps.tile([SEQ, 2, 4, 128], f32, tag=f"sc{hh}")
                            for j in range(2):
                                nc.tensor.matmul(
                                    out=sc[:, j, :, 0:SEQ],
                                    lhsT=k8[:, :, b, :],
                                    rhs=qd[:, :, bi,
                                           8 * hh + 4 * j : 8 * hh + 4 * j + 4, :],
                                    start=True, stop=True,
                                    perf_mode=mybir.MatmulPerfMode.DoubleRow)
                            nc.scalar.activation(
                                out=exps[:, 8 * hh : 8 * hh + 8, :],
                                in_=sc[:, :, :, 0:SEQ],
                                func=mybir.ActivationFunctionType.Exp,
                                bias=0.0, scale=0.25 / 64.0)
                            # attnV + denominators: per couple c two DR
                            # accumulate steps (eo), all at out base 0
                            for cc in range(2):
                                c = 2 * hh + cc
                                for kind, lhs in ((0, None), (1, ones_eo)):
                                    for eo in range(2):
                                        pr = slice(4 * c + 2 * eo,
                                                   4 * c + 2 * eo + 2)
                                        lt = (vbfs[b][:, c, eo, :, :]
                                              if kind == 0 else ones_eo[eo][:])
                                        nc.tensor.matmul(
                                            out=apscs[:, kind, c, :],
                                            lhsT=lt,
                                            rhs=exps[:, pr, :],
                                            start=(eo == 0), stop=(eo == 1),
                                            perf_mode=mybir.MatmulPerfMode.DoubleRow)
                        rdn = apool.tile([64, 4, SEQ], f32, tag="rdn")
                        nc.vector.reciprocal_approx_fast(
                            out=rdn[:], in_=apscs[:, 1, :, :])
                        nc.vector.tensor_mul(
                            out=onrm[:, :, b, :], in0=apscs[:, 0, :, :],
                            in1=rdn[:])
                        if (half, bi) in fill:
                            fill[(half, bi)]()

                emit_wc(1)
                emit_ff1(1, range(8))
                emit_ff2(1)
                x8 = x8n

            # ---- logits for this group -> DRAM bounce ----
            lgfm = apool.tile([1, GT], f32, tag="lgfm")
            for nh in range(NH):
                rr = slice(nh * 4, nh * 4 + 4)
                ps = lin_ps.tile([1, NSZ], f32, tag="lin")
                for ko in range(2):
                    nc.tensor.matmul(
                        out=ps[:],
                        lhsT=wf[:, ko : ko + 1],
                        rhs=xt[:, ko, rr],
                        start=(ko == 0), stop=(ko == 1))
                nc.scalar.copy(out=lgfm[:, nh * NSZ : (nh + 1) * NSZ], in_=ps[:])
            nc.sync.dma_start(out=lg[b0 : b0 + GR, :], in_=lgfm[:])

        # ---- epilogue: softmax + where + scatter ----
        nc.vector.tensor_add(out=lg[:], in0=lg[:], in1=mask_sb[:])
        mx = wpool.tile([RPC, 1], f32)
        nc.vector.tensor_reduce(out=mx[:], in_=lg[:], axis=mybir.AxisListType.X,
                                op=mybir.AluOpType.max, negate=True)
        pexp = wpool.tile([RPC, SEQ], f32)
        ssum = wpool.tile([RPC, 1], f32)
        nc.scalar.activation(out=pexp[:], in_=lg[:],
                             func=mybir.ActivationFunctionType.Exp,
                             bias=mx[:], scale=1.0, accum_out=ssum[:])
        rs = wpool.tile([RPC, 1], f32)
        nc.vector.reciprocal(out=rs[:], in_=ssum[:])
        props = wpool.tile([RPC, SEQ], f32)
        nc.vector.tensor_scalar_mul(out=props[:], in0=pexp[:], scalar1=rs[:])
        small = wpool.tile([RPC, SEQ], f32)
        nc.vector.tensor_scalar(out=small[:], in0=props[:], scalar1=1e-5,
                                scalar2=None, op0=mybir.AluOpType.is_le)
        pc = wpool.tile([RPC, 100], f32)
        for dst, src in ((slice(0, 50), slice(1, 51)), (slice(50, 100), slice(52, 102))):
            nc.vector.scalar_tensor_tensor(
                out=pc[:, dst], in0=small[:, src], scalar=1e-7,
                in1=props[:, src],
                op0=mybir.AluOpType.mult, op1=mybir.AluOpType.add)
        hi = wpool.tile([RPC, 100], bf16)
        nc.vector.tensor_copy(out=hi[:], in_=pc[:])
        hif = wpool.tile([RPC, 100], f32)
        nc.vector.tensor_copy(out=hif[:], in_=hi[:])
        lof = wpool.tile([RPC, 100], f32)
        nc.vector.tensor_tensor(out=lof[:], in0=pc[:], in1=hif[:],
                                op=mybir.AluOpType.subtract)
        lo = wpool.tile([RPC, 100], bf16)
        nc.vector.tensor_copy(out=lo[:], in_=lof[:])
        sc_hi = wpool.tile([RPC, OUT_W], bf16)
        sc_lo = wpool.tile([RPC, OUT_W], bf16)
        nc.gpsimd.local_scatter(out_ap=sc_hi[:], data_ap=hi[:], idxs_ap=idx_sb[:],
                                channels=RPC, num_elems=OUT_W, num_idxs=100)
        nc.gpsimd.local_scatter(out_ap=sc_lo[:], data_ap=lo[:], idxs_ap=idx_sb[:],
                                channels=RPC, num_elems=OUT_W, num_idxs=100)
        outf = wpool.tile([RPC, OUT_W], f32)
        nc.vector.tensor_tensor(out=outf[:], in0=sc_hi[:], in1=sc_lo[:],
                                op=mybir.AluOpType.add)
        nc.vector.tensor_scalar_max(out=outf[:], in0=outf[:], scalar1=1e-20)
        nc.sync.dma_start(out=out_d[:], in_=outf[:])

        a_ps.release()
        s_ps.release()
        lin_ps.release()
        espool.release()
        hdpool.release()
        vpool.release()
        apool.release()
        xpool.release()
        wpool.release()

    nc.compile()
    return nc


def get_program():
    if "nc" not in _prog_cache:
        _prog_cache["nc"] = _build_program()
    return _prog_cache["nc"]


def kernel(**inputs):
    from concourse.bass_utils import run_bass_kernel_spmd

    nc = get_program()
    w = _host_weights(inputs)

    x = np.asarray(inputs["embedded_norm_last_knn_node"], np.float32)
    perm = np.concatenate([[50], np.arange(0, 50), [101], np.arange(51, 101)])
    x_re = np.ascontiguousarray(
        x[:, perm, :].transpose(2, 0, 1).reshape(2, 128, B, SEQ).swapaxes(0, 1))
    knn_mask = np.asarray(inputs["knn_node_ninf_mask"], np.float32)
    last = np.asarray(inputs["last_unselect_list"], np.int64)
    depot = np.asarray(inputs["depot_unselect_list"], np.int64)

    mask = np.zeros((B, SEQ), np.float32)
    mask[:, 0] = -1e30
    mask[:, 51] = -1e30
    mask[:, 1:51] = knn_mask
    idx = np.concatenate([last, depot + P1], axis=1).astype(np.int16)

    in_maps = []
    for c in range(N_CORES):
        s = slice(c * RPC, (c + 1) * RPC)
        m = {"x": np.ascontiguousarray(x_re[:, :, s, :]),
             "mask": np.ascontiguousarray(mask[s]),
             "idx": np.ascontiguousarray(idx[s])}
        m.update(w)
        in_maps.append(m)

    res = run_bass_kernel_spmd(nc, in_maps, core_ids=list(range(N_CORES)))
    return np.concatenate([res.results[c]["out"] for c in range(N_CORES)], axis=0)


# revision 27
# speedup vs baseline: 1.0325x; 1.0325x over previous
"""CVRP decoder (3-layer transformer + scatter) on 8 trn2 NeuronCores.

Self-contained: hardcodes shapes/sharding for
  nn_CVRP_Decoder (B=512, SEQ=102, EMBED=256, HEADS=16, DK=16, FF=1024, L=3).

Strategy: pure data parallel over batch (64 rows/core). Per core the
whole network runs feature-major ([256->2x128 partitions, tokens]) in
groups of 8 batch rows (816 tokens); attention is row-local. All
matmuls fp8 DoubleRow (fp32 PSUM accumulate), residual stream fp32.

Attention (all fp8 DR):
 - scores: S^T for all 16 heads of a row in 2 matmuls via a
   block-diagonal q layout qd16 [128, 2, row, 16 slots, SEQ] against
   lhsT = k8 (feature-major fp8).
 - exp: ACT psum -> fp8 exps [SEQ, 16, SEQ] (2 ops/row); scores are
   tiny (|s/4| < 1) so exps are in [0.4, 2.4], perfect fp8 range.
 - attnV: per head-pair p one DR matmul, lhsT = v8d [SEQ, pair, 2, 32]
   ([V_2p | 0]j0, [0 | V_2p+1]j1), out = o block [32, SEQ] in
   apscs[32*(p%4):+32, 0, p//4]; matching ones-matmuls (shared const
   lhsT, value 64) produce the softmax denominators in
   apscs[..., 1, ...] at the SAME partitions (DVE ops require equal
   start partitions, so denominators must be born replicated).
 - normalize: rcp(denoms) + mul -> onrm fp8 [128, 2, GR, SEQ]
   (= o/8, consumed by Wc packed [128, 3, 2, 256] at x8).
Final scatter into [64, 2002] via GPSIMD local_scatter of hi/lo bf16
halves, summed in fp32.
"""

import sys

if "/opt/trn_rl_repo" not in sys.path:
    sys.path.insert(0, "/opt/trn_rl_repo")

import numpy as np
import ml_dtypes

B = 512
SEQ = 102
EMBED = 256
HEADS = 16
DK = 16
FF = 1024
LAYERS = 3
N_CORES = 8
RPC = B // N_CORES        # rows per core = 64
GR = 8                    # rows per group
GROUPS = RPC // GR        # 8
GT = GR * SEQ             # tokens per group = 816
NH = 2                    # N-halves per group (408 each)
NSZ = GT // NH            # 408
P1 = 1001
OUT_W = 2 * P1            # 2002

_prog_cache = {}


def _pack_k_major(w, ko):
    # [K, M] -> [128, ko, M] with K = ko*128 split as (ko ki)
    K, M = w.shape
    assert K == ko * 128
    return np.ascontiguousarray(w.reshape(ko, 128, M).transpose(1, 0, 2))


def _host_weights(inp):
    f32 = np.float32
    f8 = ml_dtypes.float8_e4m3

    def g(name):
        return np.asarray(inp[name], dtype=f32)

    wq, wk, wv = g("Wq"), g("Wk"), g("Wv")      # [3, 256, 256]
    wc, bc = g("Wc"), g("bc")                   # [3, 256, 256], [3, 256]
    w1, b1 = g("W1"), g("b1")                   # [3, 256, 1024], [3, 1024]
    w2, b2 = g("W2"), g("b2")                   # [3, 1024, 256], [3, 256]

    def stack_layers(ws, ko, dt):
        return np.stack([_pack_k_major(w, ko) for w in ws], axis=1).astype(dt)
        # -> [128, 3, ko, M]

    # fp8 scale folds (all exact, host-side):
    #   weights x8 into fp8; q,k carry x8 each -> exp scale /64
    #   v8d = 8v; ones value 64 -> onrm = attn/8; Wc psum = attn*wc
    #   residual bias = bc + b2 (b2 folded out of FF2)
    #   FF1 psum = 8*o1'*w1, b1' = 8*(b1 - b2@W1) corrects the folded b2
    #   FF2 psum = (8h)(8w2) -> epilogue multiplies 1/64
    out = {}
    out["wq"] = stack_layers(wq * 8, 2, f8)     # [128, 3, 2, 256]
    out["wk"] = stack_layers(wk * 8, 2, f8)
    out["wv"] = stack_layers(wv * 8, 2, f8)
    # Wc packed for the attention output layout: o lives on 64 partitions,
    # 4 couple-slots; at (partition P, slot S): hc = P//16, head
    # h = 4S + 2*(hc%2) + hc//2, dim P%16
    wc2 = np.zeros((64, 3, 4, 256), f32)
    for P in range(64):
        for S in range(4):
            hc = P // 16          # = 2*eo + j
            h = 4 * S + 2 * (hc // 2) + hc % 2
            fo = 16 * h + (P % 16)
            wc2[P, :, S, :] = wc[:, fo, :] * 8
    out["wc"] = wc2.astype(f8)                  # [64, 3, 4, 256]
    out["w1"] = stack_layers(w1 * 8, 2, f8)     # [128, 3, 2, 1024]
    out["w2"] = stack_layers(w2 * 8, 8, f8)     # [128, 3, 8, 256]
    bcb2 = bc + b2
    b1f = 8.0 * (b1 - np.einsum("ld,ldf->lf", b2, w1))
    out["bc"] = np.ascontiguousarray(
        bcb2.reshape(3, 2, 128).transpose(2, 0, 1)).astype(f32)  # [128, 3, 2]
    out["b1"] = np.ascontiguousarray(
        b1f.reshape(3, 8, 128).transpose(2, 0, 1)).astype(f32)   # [128, 3, 8]
    out["wnv"] = _pack_k_major(g("W_nv") * 8, 2).astype(f8)     # [128, 2, 256]
    out["wv2"] = _pack_k_major(g("W_v") * 8, 2).astype(f8)
    out["bnv"] = np.ascontiguousarray(
        g("b_nv").reshape(2, 128).T).astype(f32)                # [128, 2]
    out["bv2"] = np.ascontiguousarray(g("b_v").reshape(2, 128).T).astype(f32)
    out["wf"] = np.ascontiguousarray(
        g("Wf").reshape(2, 128).T).astype(f32)                  # [128, 2]
    return out


def _build_program():
    import concourse.bass as bass
    import concourse.tile as tile
    from concourse import bacc, mybir

    f32 = mybir.dt.float32
    bf16 = mybir.dt.bfloat16

    nc = bacc.Bacc("TRN2", target_bir_lowering=False, debug=False,
                   num_devices=N_CORES)

    def din(name, shape, dt=f32):
        return nc.declare_dram_parameter(name, list(shape), dt, isOutput=False)

    f8 = mybir.dt.float8e4

    x_d = din("x", [128, 2, RPC, SEQ])
    mask_d = din("mask", [RPC, SEQ])
    idx_d = din("idx", [RPC, 100], mybir.dt.int16)
    wq_d = din("wq", [128, 3, 2, 256], f8)
    wk_d = din("wk", [128, 3, 2, 256], f8)
    wv_d = din("wv", [128, 3, 2, 256], f8)
    wc_d = din("wc", [64, 3, 4, 256], f8)
    w1_d = din("w1", [128, 3, 2, 1024], f8)
    w2_d = din("w2", [128, 3, 8, 256], f8)
    bc_d = din("bc", [128, 3, 2])
    b1_d = din("b1", [128, 3, 8])
    wnv_d = din("wnv", [128, 2, 256], f8)
    wv2_d = din("wv2", [128, 2, 256], f8)
    bnv_d = din("bnv", [128, 2])
    bv2_d = din("bv2", [128, 2])
    wf_d = din("wf", [128, 2])
    out_d = nc.declare_dram_parameter("out", [RPC, OUT_W], f32, isOutput=True)

    with tile.TileContext(nc) as tc:
        wpool = tc.alloc_tile_pool(name="w", bufs=1)
        xpool = tc.alloc_tile_pool(name="x", bufs=3)
        apool = tc.alloc_tile_pool(name="a", bufs=2)
        vpool = tc.alloc_tile_pool(name="v", bufs=1)
        hdpool = tc.alloc_tile_pool(name="hd", bufs=1)
        espool = tc.alloc_tile_pool(name="es", bufs=3)
        lin_ps = tc.alloc_tile_pool(name="lps", bufs=2, space="PSUM")
        s_ps = tc.alloc_tile_pool(name="sps", bufs=1, space="PSUM")
        a_ps = tc.alloc_tile_pool(name="aps", bufs=2, space="PSUM")

        # ---- persistent weights ----
        # small first-needed tensors on sync; bulk weights on gpsimd so the
        # first group's x load isn't queued behind ~2.6MB of weight DMA
        def wtile(dram, shape, dt, tag, eng=None):
            t = wpool.tile(list(shape), dt, tag=tag)
            (eng or nc.sync).dma_start(out=t[:], in_=dram[:])
            return t

        wnv = wtile(wnv_d, [128, 2, 256], f8, "wnv")
        wv2 = wtile(wv2_d, [128, 2, 256], f8, "wv2")
        bnv = wtile(bnv_d, [128, 2], f32, "bnv")
        bv2 = wtile(bv2_d, [128, 2], f32, "bv2")
        wq = wtile(wq_d, [128, 3, 2, 256], f8, "wq", nc.gpsimd)
        wk = wtile(wk_d, [128, 3, 2, 256], f8, "wk", nc.gpsimd)
        wv = wtile(wv_d, [128, 3, 2, 256], f8, "wv", nc.gpsimd)
        wc = wtile(wc_d, [64, 3, 4, 256], f8, "wc", nc.gpsimd)
        w1 = wtile(w1_d, [128, 3, 2, 1024], f8, "w1", nc.gpsimd)
        w2 = wtile(w2_d, [128, 3, 8, 256], f8, "w2", nc.gpsimd)
        bc = wtile(bc_d, [128, 3, 2], f32, "bc", nc.gpsimd)
        b1 = wtile(b1_d, [128, 3, 8], f32, "b1", nc.gpsimd)
        wf = wtile(wf_d, [128, 2], f32, "wf", nc.gpsimd)
        mask_sb = wtile(mask_d, [RPC, SEQ], f32, "mask", nc.gpsimd)
        idx_sb = wtile(idx_d, [RPC, 100], mybir.dt.int16, "idx", nc.gpsimd)

        lg = wpool.tile([RPC, SEQ], f32)
        wfb = wpool.tile([128, 2], bf16)
        nc.vector.tensor_copy(out=wfb[:], in_=wf[:])

        # constant ones lhsTs for the denominator matmuls (couple layout:
        # head (eo, j) of a couple sits at out partitions 16*(2j+eo)); value
        # 64 so denominators land at the same partitions as the o blocks
        ones_eo = []
        for eo in range(2):
            t = wpool.tile([SEQ, 2, 64], f8, tag=f"ones{eo}")
            nc.vector.memset(t[:], 0.0)
            for j in range(2):
                m = 16 * (2 * eo + j)
                nc.vector.memset(t[:, j, m : m + 16], 64.0)
            ones_eo.append(t)

        # block-diagonal q layout [128(h%8 blocks), 2(h//8 plane), row, slot, n];
        # only the diagonal blocks are ever DMA-written, zeros persist
        qds = [hdpool.tile([128, 2, 4, HEADS, SEQ], f8, name=f"qd{i}",
                           tag=f"qd{i}") for i in range(2)]
        nc.vector.memset(qds[0][:], 0.0)
        nc.gpsimd.memset(qds[1][:], 0.0)

        # v8d ring: lhsT for attnV couples, [SEQ, couple, eo, j, m(64)];
        # V blocks rewritten per use, zero padding persists. Head
        # h = 4c + 2eo + j sits at (c, eo, j, m=16*(2eo+j)..+16).
        NV = 10
        v8ds = [vpool.tile([SEQ, 4, 2, 2, 64], f8, name=f"v8d{i}", tag=f"v8d{i}")
                for i in range(NV)]
        for i, t in enumerate(v8ds):
            (nc.vector if i % 2 else nc.gpsimd).memset(t[:], 0.0)

        for g in range(GROUPS):
            b0 = g * GR
            # ---- load x group into feature-major reordered layout ----
            xt = xpool.tile([128, 2, GR, SEQ], f32, tag="xt")
            for ko in range(2):
                nc.sync.dma_start(out=xt[:, ko], in_=x_d[:, ko, b0 : b0 + GR, :])
            # cast x8 from the raw load first (overlaps the pos projections),
            # then patch the two projected columns below
            x8 = apool.tile([128, 2, GR, SEQ], f8, tag="x8")
            for nh in range(NH):
                rr = slice(nh * 4, nh * 4 + 4)
                nc.scalar.copy(out=x8[:, :, rr], in_=xt[:, :, rr])
            # positions 0 / 51 hold raw tokens 50 / 101; project in place
            # (fp8 DoubleRow; psum = 8*(x@W) -> scale 1/8 in the bias stage)
            for w_t, b_t, pos in ((wnv, bnv, 0), (wv2, bv2, 51)):
                ps = lin_ps.tile([128, 2, GR], f32, tag="lin")
                for mo in range(2):
                    nc.tensor.matmul(
                        out=ps[:, mo, :],
                        lhsT=w_t[:, :, mo * 128 : (mo + 1) * 128],
                        rhs=x8[:, :, :, pos],
                        start=True, stop=True,
                        perf_mode=mybir.MatmulPerfMode.DoubleRow)
                for mo in range(2):
                    nc.scalar.activation(
                        out=xt[:, mo, :, pos],
                        in_=ps[:, mo, :],
                        func=mybir.ActivationFunctionType.Identity,
                        bias=b_t[:, mo : mo + 1], scale=0.125)
            for pos in (0, 51):
                nc.scalar.copy(out=x8[:, :, :, pos], in_=xt[:, :, :, pos])

            # ---- layers ----
            # later layers' x8 are cast chunk-by-chunk inside the previous
            # layer's FF2 loop so the PE never waits at the layer boundary.
            for l in range(LAYERS):
                # Q, K projections (feature-major, fp8 DoubleRow over both
                # K-halves; psum carries x8 from the weight scale)
                q8 = apool.tile([128, 2, GR, SEQ], f8, tag="q8")
                k8 = apool.tile([128, 2, GR, SEQ], f8, tag="k8")
                for nh in range(NH):
                    rr = slice(nh * 4, nh * 4 + 4)
                    for w_t, o_t in ((wq, q8), (wk, k8)):
                        for mo in range(2):
                            ps = lin_ps.tile([128, NSZ], f32, tag="lin")
                            nc.tensor.matmul(
                                out=ps[:],
                                lhsT=w_t[:, l, :, mo * 128 : (mo + 1) * 128],
                                rhs=x8[:, :, rr],
                                start=True, stop=True,
                                perf_mode=mybir.MatmulPerfMode.DoubleRow)
                            nc.vector.tensor_copy(out=o_t[:, mo, rr], in_=ps[:])

                # V token-major per row (fp8 DoubleRow; psum carries x8);
                # written straight into the v8d couple layout: head
                # (c, eo, j) -> vt[:, c, eo, j, 16*(2j+eo)]; per-eo slices
                # [:, :, eo, :, 16*eo:+16] hit exactly those positions
                vbfs = []
                for b in range(GR):
                    ps = lin_ps.tile([SEQ, 4, 2, 2, 16], f32, tag="lin")
                    nc.tensor.matmul(
                        out=ps[:],
                        lhsT=x8[:, :, b, :],
                        rhs=wv[:, l],
                        start=True, stop=True,
                        perf_mode=mybir.MatmulPerfMode.DoubleRow)
                    vt = v8ds[(g * LAYERS * GR + l * GR + b) % NV]
                    for eo in range(2):
                        for j in range(2):
                            m = 16 * (2 * eo + j)
                            if j == 0:
                                nc.vector.tensor_copy(
                                    out=vt[:, :, eo, j, m : m + 16],
                                    in_=ps[:, :, eo, j, :])
                            else:
                                nc.scalar.copy(
                                    out=vt[:, :, eo, j, m : m + 16],
                                    in_=ps[:, :, eo, j, :])
                    vbfs.append(vt)

                onrm = apool.tile([64, 4, GR, SEQ], f8, tag="onrm")
                if l == LAYERS - 1:
                    xbf = apool.tile([128, 2, GR, SEQ], bf16, tag="xbf")
                out1 = apool.tile([128, 2, GR, SEQ], f32, tag="out1")
                o18 = apool.tile([128, 2, GR, SEQ], f8, tag="o18")
                h8 = apool.tile([128, 8, GR, SEQ], f8, tag="h8", bufs=1)
                x8n = apool.tile([128, 2, GR, SEQ], f8, tag="x8")

                # linear-stack emitters (per nh column chunk); for nh=0 these
                # are interleaved into half 1's attention loop so the PE and
                # DVE/ACT stay busy through both phases
                def emit_wc(nh):
                    rr = slice(nh * 4, nh * 4 + 4)
                    for mo in range(2):
                        ps = lin_ps.tile([128, NSZ], f32, tag="lin")
                        for u in range(2):
                            nc.tensor.matmul(
                                out=ps[:],
                                lhsT=wc[:, l, 2 * u : 2 * u + 2,
                                        mo * 128 : (mo + 1) * 128],
                                rhs=onrm[:, 2 * u : 2 * u + 2, rr],
                                start=(u == 0), stop=(u == 1),
                                perf_mode=mybir.MatmulPerfMode.DoubleRow)
                        nc.vector.scalar_tensor_tensor(
                            out=out1[:, mo, rr], in0=ps[:],
                            scalar=bc[:, l, mo : mo + 1],
                            in1=xt[:, mo, rr],
                            op0=mybir.AluOpType.add, op1=mybir.AluOpType.add)
                        nc.scalar.copy(out=o18[:, mo, rr],
                                       in_=out1[:, mo, rr])

                def emit_ff1(nh, mos):
                    rr = slice(nh * 4, nh * 4 + 4)
                    for mo in mos:
                        ps = lin_ps.tile([128, NSZ], f32, tag="lin")
                        nc.tensor.matmul(
                            out=ps[:],
                            lhsT=w1[:, l, :, mo * 128 : (mo + 1) * 128],
                            rhs=o18[:, :, rr],
                            start=True, stop=True,
                            perf_mode=mybir.MatmulPerfMode.DoubleRow)
                        if mo % 2 == 0:
                            nc.scalar.activation(
                                out=h8[:, mo, rr], in_=ps[:],
                                func=mybir.ActivationFunctionType.Relu,
                                bias=b1[:, l, mo : mo + 1], scale=1.0)
                        else:
                            nc.vector.tensor_scalar(
                                out=h8[:, mo, rr], in0=ps[:],
                                scalar1=b1[:, l, mo : mo + 1], scalar2=0.0,
                                op0=mybir.AluOpType.add, op1=mybir.AluOpType.max)

                def emit_ff2(nh):
                    rr = slice(nh * 4, nh * 4 + 4)
                    for mo in range(2):
                        ps = lin_ps.tile([128, NSZ], f32, tag="lin")
                        for ko in range(4):
                            nc.tensor.matmul(
                                out=ps[:],
                                lhsT=w2[:, l, 2 * ko : 2 * ko + 2,
                                        mo * 128 : (mo + 1) * 128],
                                rhs=h8[:, 2 * ko : 2 * ko + 2, rr],
                                start=(ko == 0), stop=(ko == 3),
                                perf_mode=mybir.MatmulPerfMode.DoubleRow)
                        nc.vector.scalar_tensor_tensor(
                            out=xt[:, mo, rr], in0=ps[:],
                            scalar=1.0 / 64.0,
                            in1=out1[:, mo, rr],
                            op0=mybir.AluOpType.mult, op1=mybir.AluOpType.add)
                    if l < LAYERS - 1:
                        nc.scalar.copy(out=x8n[:, :, rr], in_=xt[:, :, rr])
                    else:
                        nc.scalar.copy(out=xbf[:, :, rr], in_=xt[:, :, rr])

                fill = {(1, 0): lambda: emit_wc(0),
                        (1, 1): lambda: emit_ff1(0, range(0, 4)),
                        (1, 2): lambda: emit_ff1(0, range(4, 8)),
                        (1, 3): lambda: emit_ff2(0)}
                dmae = [nc.sync, nc.gpsimd]
                # issue BOTH halves' qd fills up front (separate ring tiles)
                # so half 1's DMAs overlap half 0's attention
                for half in range(2):
                    qd = qds[(g * LAYERS * 2 + l * 2 + half) % 2]
                    hrr = slice(half * 4, half * 4 + 4)
                    for s in range(HEADS):
                        dmae[s % 2].dma_start(
                            out=qd[16 * (s % 8) : 16 * (s % 8) + 16, s // 8, :, s, :],
                            in_=q8[16 * (s % 8) : 16 * (s % 8) + 16, s // 8, hrr])
                for half in range(2):
                    qd = qds[(g * LAYERS * 2 + l * 2 + half) % 2]
                    for bi in range(4):
                        b = half * 4 + bi
                        exps = espool.tile([SEQ, HEADS, SEQ], f8, tag="exps")
                        apscs = a_ps.tile([64, 2, 4, SEQ], f32, tag="apscs",
                                          bufs=1)
                        for hh in range(2):
                            # psum matmul regions must stay within 2KB banks:
                            # pad the inner dim to 128 so each j-chunk is
                            # exactly one bank
                            sc = s_ps.tile([SEQ, 2, 4, 128], f32, tag=f"sc{hh}")
                            for j in range(2):
                                nc.tensor.matmul(
                                    out=sc[:, j, :, 0:SEQ],
                                    lhsT=k8[:, :, b, :],
                                    rhs=qd[:, :, bi,
                                           8 * hh + 4 * j : 8 * hh + 4 * j + 4, :],
                                    start=True, stop=True,
                                    perf_mode=mybir.MatmulPerfMode.DoubleRow)
                            nc.scalar.activation(
                                out=exps[:, 8 * hh : 8 * hh + 8, :],
                                in_=sc[:, :, :, 0:SEQ],
                                func=mybir.ActivationFunctionType.Exp,
                                bias=0.0, scale=0.25 / 64.0)
                            # attnV + denominators: per couple c two DR
                            # accumulate steps (eo), all at out base 0
                            for cc in range(2):
                                c = 2 * hh + cc
                                for kind, lhs in ((0, None), (1, ones_eo)):
                                    for eo in range(2):
                                        pr = slice(4 * c + 2 * eo,
                                                   4 * c + 2 * eo + 2)
                                        lt = (vbfs[b][:, c, eo, :, :]
                                              if kind == 0 else ones_eo[eo][:])
                                        nc.tensor.matmul(
                                            out=apscs[:, kind, c, :],
                                            lhsT=lt,
                                            rhs=exps[:, pr, :],
                                            start=(eo == 0), stop=(eo == 1),
                                            perf_mode=mybir.MatmulPerfMode.DoubleRow)
                        rdn = apool.tile([64, 4, SEQ], f32, tag="rdn")
                        nc.vector.reciprocal_approx_fast(
                            out=rdn[:], in_=apscs[:, 1, :, :])
                        nc.vector.tensor_mul(
                            out=onrm[:, :, b, :], in0=apscs[:, 0, :, :],
                            in1=rdn[:])
                        if (half, bi) in fill:
                            fill[(half, bi)]()

                emit_wc(1)
                emit_ff1(1, range(8))
                emit_ff2(1)
                x8 = x8n

            # ---- logits for this group -> DRAM bounce ----
            lgfm = apool.tile([1, GT], f32, tag="lgfm")
            for nh in range(NH):
                rr = slice(nh * 4, nh * 4 + 4)
                ps = lin_ps.tile([1, NSZ], f32, tag="lin")
                for ko in range(2):
                    nc.tensor.matmul(
                        out=ps[:],
                        lhsT=wfb[:, ko : ko + 1],
                        rhs=xbf[:, ko, rr],
                        start=(ko == 0), stop=(ko == 1))
                nc.scalar.copy(out=lgfm[:, nh * NSZ : (nh + 1) * NSZ], in_=ps[:])
            nc.sync.dma_start(out=lg[b0 : b0 + GR, :], in_=lgfm[:])

        # ---- epilogue: softmax + where + scatter ----
        nc.vector.tensor_add(out=lg[:], in0=lg[:], in1=mask_sb[:])
        mx = wpool.tile([RPC, 1], f32)
        nc.vector.tensor_reduce(out=mx[:], in_=lg[:], axis=mybir.AxisListType.X,
                                op=mybir.AluOpType.max, negate=True)
        pexp = wpool.tile([RPC, SEQ], f32)
        ssum = wpool.tile([RPC, 1], f32)
        nc.scalar.activation(out=pexp[:], in_=lg[:],
                             func=mybir.ActivationFunctionType.Exp,
                             bias=mx[:], scale=1.0, accum_out=ssum[:])
        rs = wpool.tile([RPC, 1], f32)
        nc.vector.reciprocal(out=rs[:], in_=ssum[:])
        props = wpool.tile([RPC, SEQ], f32)
        nc.vector.tensor_scalar_mul(out=props[:], in0=pexp[:], scalar1=rs[:])
        small = wpool.tile([RPC, SEQ], f32)
        nc.vector.tensor_scalar(out=small[:], in0=props[:], scalar1=1e-5,
                                scalar2=None, op0=mybir.AluOpType.is_le)
        pc = wpool.tile([RPC, 100], f32)
        for dst, src in ((slice(0, 50), slice(1, 51)), (slice(50, 100), slice(52, 102))):
            nc.vector.scalar_tensor_tensor(
                out=pc[:, dst], in0=small[:, src], scalar=1e-7,
                in1=props[:, src],
                op0=mybir.AluOpType.mult, op1=mybir.AluOpType.add)
        hi = wpool.tile([RPC, 100], bf16)
        nc.vector.tensor_copy(out=hi[:], in_=pc[:])
        hif = wpool.tile([RPC, 100], f32)
        nc.vector.tensor_copy(out=hif[:], in_=hi[:])
        lof = wpool.tile([RPC, 100], f32)
        nc.vector.tensor_tensor(out=lof[:], in0=pc[:], in1=hif[:],
                                op=mybir.AluOpType.subtract)
        lo = wpool.tile([RPC, 100], bf16)
        nc.vector.tensor_copy(out=lo[:], in_=lof[:])
        sc_hi = wpool.tile([RPC, OUT_W], bf16)
        sc_lo = wpool.tile([RPC, OUT_W], bf16)
        nc.gpsimd.local_scatter(out_ap=sc_hi[:], data_ap=hi[:], idxs_ap=idx_sb[:],
                                channels=RPC, num_elems=OUT_W, num_idxs=100)
        nc.gpsimd.local_scatter(out_ap=sc_lo[:], data_ap=lo[:], idxs_ap=idx_sb[:],
                                channels=RPC, num_elems=OUT_W, num_idxs=100)
        outf = wpool.tile([RPC, OUT_W], f32)
        nc.vector.tensor_tensor(out=outf[:], in0=sc_hi[:], in1=sc_lo[:],
                                op=mybir.AluOpType.add)
        nc.vector.tensor_scalar_max(out=outf[:], in0=outf[:], scalar1=1e-20)
        nc.sync.dma_start(out=out_d[:], in_=outf[:])

        a_ps.release()
        s_ps.release()
        lin_ps.release()
        espool.release()
        hdpool.release()
        vpool.release()
        apool.release()
        xpool.release()
        wpool.release()

    nc.compile()
    return nc


def get_program():
    if "nc" not in _prog_cache:
        _prog_cache["nc"] = _build_program()
    return _prog_cache["nc"]


def kernel(**inputs):
    from concourse.bass_utils import run_bass_kernel_spmd

    nc = get_program()
    w = _host_weights(inputs)

    x = np.asarray(inputs["embedded_norm_last_knn_node"], np.float32)
    perm = np.concatenate([[50], np.arange(0, 50), [101], np.arange(51, 101)])
    x_re = np.ascontiguousarray(
        x[:, perm, :].transpose(2, 0, 1).reshape(2, 128, B, SEQ).swapaxes(0, 1))
    knn_mask = np.asarray(inputs["knn_node_ninf_mask"], np.float32)
    last = np.asarray(inputs["last_unselect_list"], np.int64)
    depot = np.asarray(inputs["depot_unselect_list"], np.int64)

    mask = np.zeros((B, SEQ), np.float32)
    mask[:, 0] = -1e30
    mask[:, 51] = -1e30
    mask[:, 1:51] = knn_mask
    idx = np.concatenate([last, depot + P1], axis=1).astype(np.int16)

    in_maps = []
    for c in range(N_CORES):
        s = slice(c * RPC, (c + 1) * RPC)
        m = {"x": np.ascontiguousarray(x_re[:, :, s, :]),
             "mask": np.ascontiguousarray(mask[s]),
             "idx": np.ascontiguousarray(idx[s])}
        m.update(w)
        in_maps.append(m)

    res = run_bass_kernel_spmd(nc, in_maps, core_ids=list(range(N_CORES)))
    return np.concatenate([res.results[c]["out"] for c in range(N_CORES)], axis=0)
